# revision 30
# baseline (speedup 1.0000x reference)
"""Multi-head attention (B=2, S=2048, D=1024, H=16, dh=64) on 8 TRN2 NeuronCores.

Sharding: data-parallel over batch (2) x tensor-parallel over heads (4 per core).
Core c handles batch c//4 and heads [4*(c%4), 4*(c%4)+4). Each core computes a
partial output (its heads' contribution through Wo); the host sums the 4 partials
per batch and adds bo.

Schedule: the scalar engine (exp over 16.8M logits at 1 elem/cycle/lane @1.2GHz)
is the pacer, so the kernel is a single fine-grained pipeline that keeps ACT
saturated: per key-tile "slot" the PE issues QK for both head pairs, ACT exps the
previous stage, and the PE fills the rest of the slot with PV (lagging a few
slots), softmax-denominator matmuls, and interleaved k/q/v projection and Wo
output-projection work.

PE-array tiling exploits:
 - QK has contraction K=64 (head dim): the two heads of a pair sit at row tiles
   (0,0) and (64,0) and their matmuls are issued adjacently -> they execute
   concurrently (2x QK).
 - PV has M=64 (value dims): the two heads of a pair write PSUM partitions 0:64
   and 64:128 via col tiles (0,0)/(0,64) -> concurrent (2x PV).
 - Softmax denominators are M=1 matmuls against an all-ones stationary, col-tiled
   at positions (0,0/32/64/96) for the 4 heads -> all four concurrent, one PSUM
   bank, one reciprocal per chunk.

PSUM budget (8 banks): QK stages A/B [P,2,CH] = 4, PV pair accumulators = 2,
denominators = 1, shared projection/Wo bank = 1 (the lead-in alternates over the
not-yet-used attention banks to avoid evacuation stalls).
"""

import sys

if "/opt/trn_rl_repo" not in sys.path:
    sys.path.insert(0, "/opt/trn_rl_repo")

from collections import deque

import ml_dtypes
import numpy as np

import concourse.bass as bass
import concourse.mybir as mybir
import concourse.tile as tile
from concourse import bacc, bass_utils
from concourse.bass import ts

# Problem constants (hardcoded per contract)
B, S, D = 2, 2048, 1024
H, DH = 16, 64            # total heads, head dim
HC = 4                    # heads per core
DHC = HC * DH             # 256 projected dims per core
NCORES = 8
P = 128
CH = 512                  # query-chunk
NCH = S // CH             # 4
TT = S // P               # 16 key tiles
KO = D // P               # 8 contraction tiles for projections
LAG = 6                   # PV trails QK by this many key-tile slots

f32 = mybir.dt.float32
bf16 = mybir.dt.bfloat16
EXP = mybir.ActivationFunctionType.Exp

_compiled = None          # cached nc across calls
last_results = None       # BassKernelResults of the most recent run (for profiling)


def _build():
    nc = bacc.Bacc("TRN2", target_bir_lowering=False, debug=False)

    qT = nc.dram_tensor("qT", [D, S], bf16, kind="ExternalInput")
    kT = nc.dram_tensor("kT", [D, S], bf16, kind="ExternalInput")
    vT = nc.dram_tensor("vT", [D, S], bf16, kind="ExternalInput")
    wq = nc.dram_tensor("wq", [D, DHC], bf16, kind="ExternalInput")
    wk = nc.dram_tensor("wk", [D, DHC], bf16, kind="ExternalInput")
    wv = nc.dram_tensor("wv", [D, DHC], bf16, kind="ExternalInput")
    wo = nc.dram_tensor("wo", [DHC, D], bf16, kind="ExternalInput")
    bq = nc.dram_tensor("bq", [DHC], f32, kind="ExternalInput")
    bk = nc.dram_tensor("bk", [DHC], f32, kind="ExternalInput")
    bv = nc.dram_tensor("bv", [DHC], f32, kind="ExternalInput")
    out = nc.dram_tensor("out", [S, D], f32, kind="ExternalOutput")

    qT_r = qT.ap().rearrange("(ko p) s -> p ko s", p=P)
    kT_r = kT.ap().rearrange("(ko p) s -> p ko s", p=P)
    vT_r = vT.ap().rearrange("(ko p) s -> p ko s", p=P)

    with tile.TileContext(nc) as tc:
        with (
            tc.tile_pool(name="weights", bufs=1) as wpool,
            tc.tile_pool(name="acts", bufs=1) as apool,
            tc.tile_pool(name="xin", bufs=5) as xpool,
            tc.tile_pool(name="pt", bufs=8) as ptpool,
            tc.tile_pool(name="small", bufs=1) as spool,
            tc.tile_pool(name="outs", bufs=3) as opool,
            tc.tile_pool(name="psum", bufs=1, space="PSUM") as psum,
        ):
            # ---- weights / constants ----
            # DMA emission order follows first-use: k-path, q-path, v-path,
            # biases, then wo (first needed ~20 slots in).
            wq_sb = wpool.tile([P, KO, DHC], bf16, tag="wq")
            wk_sb = wpool.tile([P, KO, DHC], bf16, tag="wk")
            wv_sb = wpool.tile([P, KO, DHC], bf16, tag="wv")
            wo_sb = wpool.tile([P, DHC // P, D], bf16, tag="wo")
            nc.sync.dma_start(out=wk_sb, in_=wk.ap().rearrange("(ko p) m -> p ko m", p=P))
            nc.scalar.dma_start(out=wq_sb, in_=wq.ap().rearrange("(ko p) m -> p ko m", p=P))
            nc.gpsimd.dma_start(out=wv_sb, in_=wv.ap().rearrange("(ko p) m -> p ko m", p=P))
            bq_sb = wpool.tile([P, 2], f32, tag="bq")
            bk_sb = wpool.tile([P, 2], f32, tag="bk")
            nc.gpsimd.dma_start(out=bq_sb, in_=bq.ap().rearrange("(mo p) -> p mo", p=P))
            nc.gpsimd.dma_start(out=bk_sb, in_=bk.ap().rearrange("(mo p) -> p mo", p=P))
            bv_row = wpool.tile([P, DHC], f32, tag="bv_row")
            nc.gpsimd.dma_start(out=bv_row[0:1, :], in_=bv.ap().rearrange("(a d) -> a d", a=1))
            bv_bc = wpool.tile([P, DHC], f32, tag="bv_bc")
            nc.gpsimd.partition_broadcast(bv_bc, bv_row[0:1, :])
            ones_sb = wpool.tile([P, 1], bf16, tag="ones")
            nc.vector.memset(ones_sb, 1.0)
            # zero operands for the PSUM-clearing matmul that opens each shared
            # accumulation bank (PV pairs / denominators share banks between
            # chains, so only the clear carries start=True).
            zero_sb = wpool.tile([P, CH], bf16, tag="zero")
            nc.vector.memset(zero_sb, 0.0)

            # ---- activations ----
            # q^T/k^T: [P, pair, S]; projected dim r of pair m sits at
            # (r % 128, m): head 2m at partitions 0:64, head 2m+1 at 64:128.
            q_sb = apool.tile([P, 2, S], bf16, tag="q")
            k_sb = apool.tile([P, 2, S], bf16, tag="k")
            # v natural: [P(key in tile), tt, h*dh]
            v_sb = apool.tile([P, TT, DHC], bf16, tag="v")
            v_heads = v_sb[:, :, :].rearrange("p tt (h c) -> p tt h c", c=DH)
            # attn^T accumulator, same layout as q_sb
            attn_sb = apool.tile([P, 2, S], bf16, tag="attn")

            # ---- DMA + projection-group emitters ----
            # x activations arrive as chunk-PAIR tiles [P, KO, 2*CH]: 2KB DRAM
            # lines run the DMA engines at ~2x the 1KB-line rate.
            xk2 = [None, None]
            xq2 = [None, None]
            xv2 = [None, None]

            def dma_k2(p, eng=None):
                xk2[p] = xpool.tile([P, KO, 2 * CH], bf16, tag="x", name=f"xk2_{p}")
                (eng or nc.sync).dma_start(out=xk2[p], in_=kT_r[:, :, ts(p, 2 * CH)])

            def dma_q2(p, eng=None):
                xq2[p] = xpool.tile([P, KO, 2 * CH], bf16, tag="x", name=f"xq2_{p}")
                (eng or nc.sync).dma_start(out=xq2[p], in_=qT_r[:, :, ts(p, 2 * CH)])

            def dma_v2(p, eng=None):
                xv2[p] = xpool.tile([P, KO, 2 * CH], bf16, tag="x", name=f"xv2_{p}")
                (eng or nc.sync).dma_start(out=xv2[p], in_=vT_r[:, :, ts(p, 2 * CH)])

            def xk_ap(c):
                return xk2[c // 2][:, :, ts(c % 2, CH)]

            def xq_ap(c):
                return xq2[c // 2][:, :, ts(c % 2, CH)]

            def kq_group(c, m, dst, w_sb, b_sb, x, ptag, uid):
                # one projection group: [P, CH] of pair m for chunk c
                ps = psum.tile([P, CH], f32, tag=ptag, name=f"pj_{uid}_{m}")
                for ko in range(KO):
                    nc.tensor.matmul(ps, w_sb[:, ko, ts(m, P)], x[:, ko, :],
                                     start=(ko == 0), stop=(ko == KO - 1))
                nc.vector.tensor_scalar_add(out=dst[:, m, ts(c, CH)], in0=ps,
                                            scalar1=b_sb[:, m : m + 1])

            def v_group(c, th):
                tt = c * (CH // P) + th
                ps = psum.tile([P, DHC], f32, tag="mx", name=f"pv_proj_{tt}")
                xv_c = xv2[c // 2]
                for ko in range(KO):
                    nc.tensor.matmul(ps, xv_c[:, ko, ts((c % 2) * (CH // P) + th, P)],
                                     wv_sb[:, ko, :],
                                     start=(ko == 0), stop=(ko == KO - 1))
                nc.vector.tensor_add(out=v_sb[:, tt, :], in0=ps, in1=bv_bc)

            def wo_group(c, st, n, ptag="mx"):
                stg = c * (CH // P) + st
                pw = psum.tile([P, 512], f32, tag=ptag, name=f"wo_{stg}_{n}")
                for ko in range(2):
                    nc.tensor.matmul(pw, attn_sb[:, ko, ts(stg, P)],
                                     wo_sb[:, ko, ts(n, 512)],
                                     start=(ko == 0), stop=(ko == 1))
                ot = opool.tile([P, 512], f32, tag="ot", name=f"ot_{stg}_{n}")
                nc.vector.tensor_copy(out=ot, in_=pw)
                nc.sync.dma_start(out=out.ap()[ts(stg, P), ts(n, 512)], in_=ot)

            # ---- filler queue: (cost_cycles, emit_fn) ----
            fillers = deque()

            def pop_fillers(budget):
                spent = 0
                while fillers and spent < budget:
                    cost, fn = fillers.popleft()
                    fn()
                    spent += cost

            # ---- lead-in ----
            # One 2MB x DMA per tensor-half, split across the three DMA issue
            # queues so k/q/v stream in parallel: sync carries wk+xk01,
            # scalar (ACT, idle pre-stream) carries wq+xq01, gpsimd (SWDGE)
            # carries everything else.
            dma_k2(0)                    # sync (wk DMA was emitted ahead of it)
            dma_q2(0, eng=nc.scalar)
            dma_v2(0, eng=nc.gpsimd)
            dma_k2(1, eng=nc.gpsimd)
            # wo is first needed ~20 slots in; keep it behind the x DMAs.
            nc.gpsimd.dma_start(out=wo_sb, in_=wo.ap().rearrange("(ko p) n -> p ko n", p=P))
            # Warm the PE HAM clock gate during the initial DMA wait: ~4.5us of
            # dependency-free matmuls (zero x zero) so the real lead-in runs at
            # 2.4GHz instead of 1.2.
            warm_ps = psum.tile([P, 2, CH], f32, tag="qk0", name="warm")
            for w in range(10):
                nc.tensor.matmul(warm_ps[:, 0, :], zero_sb[:, 0:P], zero_sb,
                                 start=True, stop=True)
            # k/q for chunk 0 on the four attention banks (not yet used) so the
            # four groups don't serialize on one bank's evacuation.
            kq_group(0, 0, k_sb, wk_sb, bk_sb, xk_ap(0), "mx", "k0")
            kq_group(0, 1, k_sb, wk_sb, bk_sb, xk_ap(0), "dn", "k0")
            kq_group(0, 0, q_sb, wq_sb, bq_sb, xq_ap(0), "pv0", "q0")
            kq_group(0, 1, q_sb, wq_sb, bq_sb, xq_ap(0), "pv1", "q0")

            # Remaining projections become filler pieces (~2048 PE cycles each):
            # k/q groups split into two half-groups sharing one PSUM tile.
            def mk_kq_halves(c, m, dst, w_sb, b_sb, xap, uid, pre=None):
                state = {}

                def fn1():
                    if pre is not None:
                        pre()
                    ps = psum.tile([P, CH], f32, tag="mx", name=f"pj_{uid}_{m}")
                    state["ps"] = ps
                    x = xap(c)
                    for ko in range(KO // 2):
                        nc.tensor.matmul(ps, w_sb[:, ko, ts(m, P)], x[:, ko, :],
                                         start=(ko == 0), stop=False)

                def fn2():
                    ps = state["ps"]
                    x = xap(c)
                    for ko in range(KO // 2, KO):
                        nc.tensor.matmul(ps, w_sb[:, ko, ts(m, P)], x[:, ko, :],
                                         start=False, stop=(ko == KO - 1))
                    nc.vector.tensor_scalar_add(out=dst[:, m, ts(c, CH)], in0=ps,
                                                scalar1=b_sb[:, m : m + 1])

                return (2048, fn1), (2048, fn2)

            def mk_v(c, th, pre=None):
                def fn():
                    if pre is not None:
                        pre()
                    v_group(c, th)
                return (2048, fn)

            def kq_pieces(c, m, dst, w_sb, b_sb, xap, uid, pre=None):
                return mk_kq_halves(c, m, dst, w_sb, b_sb, xap, uid, pre=pre)

            k1a = kq_pieces(1, 0, k_sb, wk_sb, bk_sb, xk_ap, "k1")
            k1b = kq_pieces(1, 1, k_sb, wk_sb, bk_sb, xk_ap, "k1")
            k2a = kq_pieces(2, 0, k_sb, wk_sb, bk_sb, xk_ap, "k2")
            k2b = kq_pieces(2, 1, k_sb, wk_sb, bk_sb, xk_ap, "k2")
            k3a = kq_pieces(3, 0, k_sb, wk_sb, bk_sb, xk_ap, "k3")
            k3b = kq_pieces(3, 1, k_sb, wk_sb, bk_sb, xk_ap, "k3")
            q1a = kq_pieces(1, 0, q_sb, wq_sb, bq_sb, xq_ap, "q1")
            q1b = kq_pieces(1, 1, q_sb, wq_sb, bq_sb, xq_ap, "q1")
            q2a = kq_pieces(2, 0, q_sb, wq_sb, bq_sb, xq_ap, "q2")
            q2b = kq_pieces(2, 1, q_sb, wq_sb, bq_sb, xq_ap, "q2")
            q3a = kq_pieces(3, 0, q_sb, wq_sb, bq_sb, xq_ap, "q3")
            q3b = kq_pieces(3, 1, q_sb, wq_sb, bq_sb, xq_ap, "q3")
            v_pc = {}
            for c in range(NCH):
                for th in range(4):
                    pre = None
                    if c == 0 and th == 2:
                        pre = lambda: dma_q2(1, eng=nc.gpsimd)
                    if c == 1 and th == 0:
                        pre = lambda: dma_v2(1, eng=nc.gpsimd)
                    v_pc[(c, th)] = mk_v(c, th, pre=pre)

            # Interleave so v-block tt lands by slot tt+LAG, k chunk cc by slot
            # 4*cc, q chunk cc by slot 16*cc.
            order = [
                v_pc[(0, 0)], k1a[0], k1a[1],
                v_pc[(0, 1)], k1b[0], k1b[1],
                v_pc[(0, 2)], k2a[0], k2a[1],
                v_pc[(0, 3)], k2b[0], k2b[1],
                v_pc[(1, 0)], k3a[0], k3a[1],
                v_pc[(1, 1)], k3b[0], k3b[1],
                v_pc[(1, 2)], v_pc[(1, 3)],
                q1a[0], q1a[1], q1b[0], q1b[1],
                v_pc[(2, 0)], v_pc[(2, 1)], v_pc[(2, 2)], v_pc[(2, 3)],
                v_pc[(3, 0)], v_pc[(3, 1)], v_pc[(3, 2)], v_pc[(3, 3)],
                q2a[0], q2a[1], q2b[0], q2b[1],
                q3a[0], q3a[1], q3b[0], q3b[1],
            ]
            fillers.extend(order)

            # ---- attention stream ----
            # Global slot stream: chunk c contributes 16 QK/exp slots; PV for
            # (c, tt) is emitted LAG slots later (spilling into the next
            # chunk's slots); normalization at each chunk boundary.
            pts = {}       # (chunk, tt, pair) -> pts tile
            pv_ps = {}     # (chunk, pair) -> PSUM accumulator
            dn_ps = {}     # chunk -> denominator PSUM

            def emit_qk(c, tt):
                csl = ts(c, CH)
                for pm in range(2):
                    st = psum.tile([P, 2, CH], f32, tag=f"qk{pm}", name=f"qk_{c}_{tt}_{pm}")
                    for j in range(2):
                        base = DH * j
                        nc.tensor.matmul(
                            st[:, j, :],
                            k_sb[base : base + DH, pm, ts(tt, P)],
                            q_sb[base : base + DH, pm, csl],
                            start=True, stop=True,
                        )
                    pt = ptpool.tile([P, 2, CH], bf16, tag=f"pt{pm}", name=f"pt_{c}_{tt}_{pm}")
                    nc.scalar.activation(out=pt, in_=st, func=EXP)
                    pts[(c, tt, pm)] = pt

            def emit_pv(c, tt):
                if tt == 0:
                    # open each shared bank with a full-width zeroing matmul
                    # (the only start=True): the per-head chains then ride its
                    # has_written bits with start=False.
                    for pm in range(2):
                        po = psum.tile([P, CH], f32, tag=f"pv{pm}", name=f"pv_{c}_{pm}")
                        pv_ps[(c, pm)] = po
                        nc.tensor.matmul(po, zero_sb[:, 0:P], zero_sb,
                                         start=True, stop=False, skip_group_check=True)
                    dn = psum.tile([P, CH], f32, tag="dn", name=f"dn_{c}")
                    dn_ps[c] = dn
                    nc.tensor.matmul(dn, zero_sb[:, 0:P], zero_sb,
                                     start=True, stop=False, skip_group_check=True)
                for pm in range(2):
                    pt = pts[(c, tt, pm)]
                    po = pv_ps[(c, pm)]
                    for j in range(2):
                        # stop on BOTH chains at the last tile: the group
                        # bookkeeping is per-partition, and each chain's stop
                        # closes only its own partition range.
                        nc.tensor.matmul(
                            po[DH * j : DH * j + DH, :],
                            v_heads[:, tt, 2 * pm + j, :],
                            pt[:, j, :],
                            start=False,
                            stop=(tt == TT - 1),
                            skip_group_check=True,
                        )
                dn = dn_ps[c]
                for pm in range(2):
                    pt = pts[(c, tt, pm)]
                    for j in range(2):
                        h = 2 * pm + j
                        nc.tensor.matmul(
                            dn[32 * h : 32 * h + 1, :],
                            ones_sb,
                            pt[:, j, :],
                            start=False,
                            stop=False,
                            skip_group_check=True,
                            tile_position=(0, 32 * h),
                        )
                if tt == TT - 1:
                    # the denominator chains only touch partitions 0/32/64/96;
                    # a full-width zero-add closes the whole bank's group.
                    nc.tensor.matmul(dn, zero_sb[:, 0:P], zero_sb,
                                     start=False, stop=True, skip_group_check=True)

            def emit_norm(c, tail=False):
                csl = ts(c, CH)
                # fast copies free the PV/denominator banks immediately; the
                # slow reciprocal then runs off the critical path from SBUF.
                tmp = {}
                for pm in range(2):
                    t = spool.tile([P, CH], f32, tag=f"tmp{pm}", name=f"tmp_{c}_{pm}")
                    nc.vector.tensor_copy(out=t, in_=pv_ps[(c, pm)])
                    tmp[pm] = t
                rec = spool.tile([P, CH], f32, tag="rec", name=f"rec_{c}")
                dnc = spool.tile([P, CH], f32, tag="dnc", name=f"dnc_{c}")
                nc.vector.tensor_copy(out=dnc[0:97, :], in_=dn_ps[c][0:97, :])
                nc.vector.reciprocal(out=rec[0:97, :], in_=dnc[0:97, :])
                # move the three off-partition-0 reciprocal rows to partition 0
                # (SBUF->SBUF DMA remaps partitions; gpsimd broadcast reads
                # partition 0 only)
                rrow = [rec[0:1, :]]
                for h in range(1, 4):
                    r = spool.tile([1, CH], f32, tag=f"r{h}", name=f"r_{c}_{h}")
                    nc.sync.dma_start(out=r, in_=rec[32 * h : 32 * h + 1, :])
                    rrow.append(r[0:1, :])
                # HW partition_broadcast only writes from partition base 0:
                # broadcast each head's reciprocal across all 128 partitions
                # and multiply per head with partition-aligned slices.
                for pm in range(2):
                    for j in range(2):
                        b = spool.tile([P, CH], f32, tag=f"bc{pm}{j}", name=f"bc_{c}_{pm}{j}")
                        nc.gpsimd.partition_broadcast(b, rrow[2 * pm + j])
                        sl = slice(DH * j, DH * (j + 1))
                        nc.vector.tensor_mul(out=attn_sb[sl, pm, csl],
                                             in0=tmp[pm][sl, :], in1=b[sl, :])

            slot = [0]

            def run_slot(qk_ct, pv_ct, norm_c=None):
                # chunk 0 drains most of the projection filler (PE-bound);
                # later chunks pace one piece per slot under the ACT pacer.
                # The two slots before each chunk boundary pop nothing so the
                # DVE queue is clear for the bank-freeing copies of emit_norm
                # (a backed-up copy stalls the next chunk's PV clears and, via
                # the in-order PE queue, starves ACT).
                if slot[0] < TT:
                    pop_fillers(2000)
                if qk_ct is not None:
                    emit_qk(*qk_ct)
                if pv_ct is not None:
                    emit_pv(*pv_ct)
                if norm_c is not None:
                    emit_norm(norm_c)
                    for st_ in range(CH // P):
                        for n_ in range(2):
                            fillers.append((1024, (lambda cc, ss, nn: lambda: wo_group(cc, ss, nn))(norm_c, st_, n_)))
                boundary = slot[0] >= TT and slot[0] % TT in (3, 4)
                if not boundary:
                    if slot[0] < TT:
                        pop_fillers(2000 if slot[0] % 2 == 0 else 1)
                    else:
                        pop_fillers(1)
                slot[0] += 1

            stream = []  # (chunk, tt) in QK order
            for c in range(NCH):
                for tt in range(TT):
                    stream.append((c, tt))

            for i, (c, tt) in enumerate(stream):
                pv_i = i - LAG
                pv_ct = stream[pv_i] if pv_i >= 0 else None
                norm_c = None
                if pv_ct is not None and pv_ct[1] == TT - 1 and pv_ct[0] < NCH - 1:
                    norm_c = pv_ct[0]
                run_slot((c, tt), pv_ct, norm_c)

            # tail: trailing PV slots, keep-warm matmuls so the HAM gate stays
            # hot through the final norm chain, then the last chunk's norm and
            # Wo rotated over the now-free banks.
            for i in range(len(stream), len(stream) + LAG):
                pv_ct = stream[i - LAG]
                run_slot(None, pv_ct)
            emit_norm(NCH - 1, tail=True)
            warm2 = psum.tile([P, 2, CH], f32, tag="qk0", name="warm2")
            for w in range(24):
                nc.tensor.matmul(warm2[:, 0, 0:256], zero_sb[:, 0:P], zero_sb[:, 0:256],
                                 start=True, stop=True)
            while fillers:
                pop_fillers(10**9)
            tail_tags = ["mx", "qk1", "pv0", "pv1", "dn"]
            for idx, (st, n) in enumerate((s, n) for s in range(CH // P) for n in range(2)):
                wo_group(NCH - 1, st, n, ptag=tail_tags[idx % len(tail_tags)])

    nc.finalize()
    return nc


def kernel(**inputs):
    global _compiled, last_results
    if _compiled is None:
        _compiled = _build()
    nc = _compiled

    query = np.asarray(inputs["query"], np.float32)
    key = np.asarray(inputs["key"], np.float32)
    value = np.asarray(inputs["value"], np.float32)
    Wq = np.asarray(inputs["Wq"], np.float32)
    Wk = np.asarray(inputs["Wk"], np.float32)
    Wv = np.asarray(inputs["Wv"], np.float32)
    Wo = np.asarray(inputs["Wo"], np.float32)
    bq_f = np.asarray(inputs["bq"], np.float32)
    bk_f = np.asarray(inputs["bk"], np.float32)
    bv_f = np.asarray(inputs["bv"], np.float32)
    bo_f = np.asarray(inputs["bo"], np.float32)

    bf = ml_dtypes.bfloat16
    scale = 1.0 / np.sqrt(np.float32(DH))
    qT = [np.ascontiguousarray(query[b].T).astype(bf) for b in range(B)]
    kT = [np.ascontiguousarray(key[b].T).astype(bf) for b in range(B)]
    vT = [np.ascontiguousarray(value[b].T).astype(bf) for b in range(B)]

    in_maps = []
    for c in range(NCORES):
        b = c // 4
        sh = c % 4
        sl = slice(DHC * sh, DHC * (sh + 1))
        in_maps.append({
            "qT": qT[b], "kT": kT[b], "vT": vT[b],
            "wq": (Wq[:, sl] * scale).astype(bf),
            "wk": np.ascontiguousarray(Wk[:, sl]).astype(bf),
            "wv": np.ascontiguousarray(Wv[:, sl]).astype(bf),
            "wo": np.ascontiguousarray(Wo[sl, :]).astype(bf),
            "bq": np.ascontiguousarray(bq_f[sl]) * scale,
            "bk": np.ascontiguousarray(bk_f[sl]),
            "bv": np.ascontiguousarray(bv_f[sl]),
        })

    res = bass_utils.run_bass_kernel_spmd(nc, in_maps, core_ids=list(range(NCORES)))
    last_results = res

    final = np.empty((B, S, D), np.float32)
    for b in range(B):
        acc = res.results[4 * b]["out"].astype(np.float32)
        for sh in range(1, 4):
            acc = acc + res.results[4 * b + sh]["out"]
        final[b] = acc + bo_f
    return final


# revision 31
# speedup vs baseline: 1.1963x; 1.1963x over previous
"""Multi-head attention (B=2, S=2048, D=1024, H=16, dh=64) on 8 TRN2 NeuronCores.

Sharding: data-parallel over batch (2) x tensor-parallel over heads (4 per core).
Core c handles batch c//4 and heads [4*(c%4), 4*(c%4)+4). Each core computes a
partial output (its heads' contribution through Wo); the host sums the 4 partials
per batch and adds bo.

Schedule: the scalar engine (exp over 16.8M logits at 1 elem/cycle/lane @1.2GHz)
is the pacer, so the kernel is a single fine-grained pipeline that keeps ACT
saturated: per key-tile "slot" the PE issues QK for both head pairs, ACT exps the
previous stage, and the PE fills the rest of the slot with PV (lagging a few
slots), softmax-denominator matmuls, and interleaved k/q/v projection and Wo
output-projection work.

PE-array tiling exploits:
 - QK has contraction K=64 (head dim): the two heads of a pair sit at row tiles
   (0,0) and (64,0) and their matmuls are issued adjacently -> they execute
   concurrently (2x QK).
 - PV has M=64 (value dims): the two heads of a pair write PSUM partitions 0:64
   and 64:128 via col tiles (0,0)/(0,64) -> concurrent (2x PV).
 - Softmax denominators are M=1 matmuls against an all-ones stationary, col-tiled
   at positions (0,0/32/64/96) for the 4 heads -> all four concurrent, one PSUM
   bank, one reciprocal per chunk.

PSUM budget (8 banks): QK stages A/B [P,2,CH] = 4, PV pair accumulators = 2,
denominators = 1, shared projection/Wo bank = 1 (the lead-in alternates over the
not-yet-used attention banks to avoid evacuation stalls).
"""

import sys

if "/opt/trn_rl_repo" not in sys.path:
    sys.path.insert(0, "/opt/trn_rl_repo")

from collections import deque

import ml_dtypes
import numpy as np

import concourse.bass as bass
import concourse.mybir as mybir
import concourse.tile as tile
from concourse import bacc, bass_utils
from concourse.bass import ts

# Problem constants (hardcoded per contract)
B, S, D = 2, 2048, 1024
H, DH = 16, 64            # total heads, head dim
HC = 4                    # heads per core
DHC = HC * DH             # 256 projected dims per core
NCORES = 8
P = 128
CH = 512                  # query-chunk
NCH = S // CH             # 4
TT = S // P               # 16 key tiles
KO = D // P               # 8 contraction tiles for projections
LAG = 6                   # PV trails QK by this many key-tile slots

f32 = mybir.dt.float32
bf16 = mybir.dt.bfloat16
EXP = mybir.ActivationFunctionType.Exp

_compiled = None          # cached nc across calls
last_results = None       # BassKernelResults of the most recent run (for profiling)


def _build():
    nc = bacc.Bacc("TRN2", target_bir_lowering=False, debug=False)

    qT = nc.dram_tensor("qT", [D, S], bf16, kind="ExternalInput")
    kT = nc.dram_tensor("kT", [D, S], bf16, kind="ExternalInput")
    vT = nc.dram_tensor("vT", [D, S], bf16, kind="ExternalInput")
    wq = nc.dram_tensor("wq", [D, DHC], bf16, kind="ExternalInput")
    wk = nc.dram_tensor("wk", [D, DHC], bf16, kind="ExternalInput")
    wv = nc.dram_tensor("wv", [D, DHC], bf16, kind="ExternalInput")
    wo = nc.dram_tensor("wo", [DHC, D], bf16, kind="ExternalInput")
    bq = nc.dram_tensor("bq", [DHC], f32, kind="ExternalInput")
    bk = nc.dram_tensor("bk", [DHC], f32, kind="ExternalInput")
    bv = nc.dram_tensor("bv", [DHC], f32, kind="ExternalInput")
    out = nc.dram_tensor("out", [S, D], f32, kind="ExternalOutput")

    qT_r = qT.ap().rearrange("(ko p) s -> p ko s", p=P)
    kT_r = kT.ap().rearrange("(ko p) s -> p ko s", p=P)
    vT_r = vT.ap().rearrange("(ko p) s -> p ko s", p=P)

    with tile.TileContext(nc) as tc:
        with (
            tc.tile_pool(name="weights", bufs=1) as wpool,
            tc.tile_pool(name="acts", bufs=1) as apool,
            tc.tile_pool(name="xin", bufs=11) as xpool,
            tc.tile_pool(name="pt", bufs=7) as ptpool,
            tc.tile_pool(name="small", bufs=1) as spool,
            tc.tile_pool(name="outs", bufs=2) as opool,
            tc.tile_pool(name="psum", bufs=1, space="PSUM") as psum,
        ):
            # ---- weights / constants ----
            # DMA emission order follows first-use: k-path, q-path, v-path,
            # biases, then wo (first needed ~20 slots in).
            wq_sb = wpool.tile([P, KO, DHC], bf16, tag="wq")
            wk_sb = wpool.tile([P, KO, DHC], bf16, tag="wk")
            wv_sb = wpool.tile([P, KO, DHC], bf16, tag="wv")
            wo_sb = wpool.tile([P, DHC // P, D], bf16, tag="wo")
            nc.sync.dma_start(out=wk_sb, in_=wk.ap().rearrange("(ko p) m -> p ko m", p=P))
            nc.scalar.dma_start(out=wq_sb, in_=wq.ap().rearrange("(ko p) m -> p ko m", p=P))
            nc.gpsimd.dma_start(out=wv_sb, in_=wv.ap().rearrange("(ko p) m -> p ko m", p=P))
            bq_sb = wpool.tile([P, 2], f32, tag="bq")
            bk_sb = wpool.tile([P, 2], f32, tag="bk")
            nc.gpsimd.dma_start(out=bq_sb, in_=bq.ap().rearrange("(mo p) -> p mo", p=P))
            nc.gpsimd.dma_start(out=bk_sb, in_=bk.ap().rearrange("(mo p) -> p mo", p=P))
            bv_row = wpool.tile([P, DHC], f32, tag="bv_row")
            nc.gpsimd.dma_start(out=bv_row[0:1, :], in_=bv.ap().rearrange("(a d) -> a d", a=1))
            bv_bc = wpool.tile([P, DHC], f32, tag="bv_bc")
            nc.gpsimd.partition_broadcast(bv_bc, bv_row[0:1, :])
            ones_sb = wpool.tile([P, 1], bf16, tag="ones")
            nc.vector.memset(ones_sb, 1.0)
            # zero operands for the PSUM-clearing matmul that opens each shared
            # accumulation bank (PV pairs / denominators share banks between
            # chains, so only the clear carries start=True).
            zero_sb = wpool.tile([P, CH], bf16, tag="zero")
            nc.vector.memset(zero_sb, 0.0)

            # ---- activations ----
            # q^T/k^T: [P, pair, S]; projected dim r of pair m sits at
            # (r % 128, m): head 2m at partitions 0:64, head 2m+1 at 64:128.
            q_sb = apool.tile([P, 2, S], bf16, tag="q")
            k_sb = apool.tile([P, 2, S], bf16, tag="k")
            # v natural: [P(key in tile), tt, h*dh]
            v_sb = apool.tile([P, TT, DHC], bf16, tag="v")
            v_heads = v_sb[:, :, :].rearrange("p tt (h c) -> p tt h c", c=DH)
            # attn^T accumulator, same layout as q_sb
            attn_sb = apool.tile([P, 2, S], bf16, tag="attn")

            # ---- DMA + projection-group emitters ----
            xk = [None] * NCH
            xq = [None] * NCH
            xv = [None] * NCH

            def dma_k(c, eng=None):
                xk[c] = xpool.tile([P, KO, CH], bf16, tag="x", name=f"xk{c}")
                (eng or nc.sync).dma_start(out=xk[c], in_=kT_r[:, :, ts(c, CH)])

            def dma_q(c, eng=None):
                xq[c] = xpool.tile([P, KO, CH], bf16, tag="x", name=f"xq{c}")
                (eng or nc.sync).dma_start(out=xq[c], in_=qT_r[:, :, ts(c, CH)])

            def dma_v(c, eng=None):
                xv[c] = xpool.tile([P, KO, CH], bf16, tag="x", name=f"xv{c}")
                (eng or nc.sync).dma_start(out=xv[c], in_=vT_r[:, :, ts(c, CH)])

            def xk_ap(c):
                return xk[c]

            def xq_ap(c):
                return xq[c]

            def kq_group(c, m, dst, w_sb, b_sb, x, ptag, uid):
                # one projection group: [P, CH] of pair m for chunk c
                ps = psum.tile([P, CH], f32, tag=ptag, name=f"pj_{uid}_{m}")
                for ko in range(KO):
                    nc.tensor.matmul(ps, w_sb[:, ko, ts(m, P)], x[:, ko, :],
                                     start=(ko == 0), stop=(ko == KO - 1))
                nc.vector.tensor_scalar_add(out=dst[:, m, ts(c, CH)], in0=ps,
                                            scalar1=b_sb[:, m : m + 1])

            def v_group(c, th):
                tt = c * (CH // P) + th
                ps = psum.tile([P, DHC], f32, tag="mx", name=f"pv_proj_{tt}")
                for ko in range(KO):
                    nc.tensor.matmul(ps, xv[c][:, ko, ts(th, P)], wv_sb[:, ko, :],
                                     start=(ko == 0), stop=(ko == KO - 1))
                nc.vector.tensor_add(out=v_sb[:, tt, :], in0=ps, in1=bv_bc)

            def wo_group(c, st, n, ptag="mx"):
                stg = c * (CH // P) + st
                pw = psum.tile([P, 512], f32, tag=ptag, name=f"wo_{stg}_{n}")
                for ko in range(2):
                    nc.tensor.matmul(pw, attn_sb[:, ko, ts(stg, P)],
                                     wo_sb[:, ko, ts(n, 512)],
                                     start=(ko == 0), stop=(ko == 1))
                ot = opool.tile([P, 512], f32, tag="ot", name=f"ot_{stg}_{n}")
                nc.vector.tensor_copy(out=ot, in_=pw)
                nc.sync.dma_start(out=out.ap()[ts(stg, P), ts(n, 512)], in_=ot)

            # ---- filler queue: (cost_cycles, emit_fn) ----
            fillers = deque()

            def pop_fillers(budget):
                spent = 0
                while fillers and spent < budget:
                    cost, fn = fillers.popleft()
                    fn()
                    spent += cost

            # ---- lead-in ----
            # All x DMAs kick up-front in deadline order on the two HWDGE
            # queues (~190GB/s each measured): sync carries wk + k + v, the
            # scalar queue (idle pre-stream) carries wq + q. SWDGE (gpsimd)
            # only moves the small weights so it never contends for HBM in
            # the critical window.
            dma_k(0)
            dma_q(0, eng=nc.scalar)
            dma_k(1)
            dma_q(1, eng=nc.scalar)
            dma_v(0)
            dma_q(2, eng=nc.scalar)
            dma_k(2)
            dma_k(3)
            dma_q(3, eng=nc.scalar)
            dma_v(1)
            # wo is first needed ~20 slots in; keep it behind the x DMAs.
            nc.gpsimd.dma_start(out=wo_sb, in_=wo.ap().rearrange("(ko p) n -> p ko n", p=P))
            # Warm the PE HAM clock gate during the initial DMA wait: ~4.5us of
            # dependency-free matmuls (zero x zero) so the real lead-in runs at
            # 2.4GHz instead of 1.2.
            warm_ps = psum.tile([P, 2, CH], f32, tag="qk0", name="warm")
            for w in range(10):
                nc.tensor.matmul(warm_ps[:, 0, :], zero_sb[:, 0:P], zero_sb,
                                 start=True, stop=True)
            # k/q for chunk 0 on the four attention banks (not yet used) so the
            # four groups don't serialize on one bank's evacuation.
            kq_group(0, 0, k_sb, wk_sb, bk_sb, xk[0], "mx", "k0")
            kq_group(0, 1, k_sb, wk_sb, bk_sb, xk[0], "dn", "k0")
            kq_group(0, 0, q_sb, wq_sb, bq_sb, xq[0], "pv0", "q0")
            kq_group(0, 1, q_sb, wq_sb, bq_sb, xq[0], "pv1", "q0")

            # Remaining projections become filler pieces (~2048 PE cycles each):
            # k/q groups split into two half-groups sharing one PSUM tile.
            def mk_kq_halves(c, m, dst, w_sb, b_sb, xap, uid, pre=None):
                state = {}

                def fn1():
                    if pre is not None:
                        pre()
                    ps = psum.tile([P, CH], f32, tag="mx", name=f"pj_{uid}_{m}")
                    state["ps"] = ps
                    x = xap(c)
                    for ko in range(KO // 2):
                        nc.tensor.matmul(ps, w_sb[:, ko, ts(m, P)], x[:, ko, :],
                                         start=(ko == 0), stop=False)

                def fn2():
                    ps = state["ps"]
                    x = xap(c)
                    for ko in range(KO // 2, KO):
                        nc.tensor.matmul(ps, w_sb[:, ko, ts(m, P)], x[:, ko, :],
                                         start=False, stop=(ko == KO - 1))
                    nc.vector.tensor_scalar_add(out=dst[:, m, ts(c, CH)], in0=ps,
                                                scalar1=b_sb[:, m : m + 1])

                return (2048, fn1), (2048, fn2)

            def mk_v(c, th, pre=None):
                def fn():
                    if pre is not None:
                        pre()
                    v_group(c, th)
                return (2048, fn)

            def kq_pieces(c, m, dst, w_sb, b_sb, xap, uid, pre=None):
                return mk_kq_halves(c, m, dst, w_sb, b_sb, xap, uid, pre=pre)

            k1a = kq_pieces(1, 0, k_sb, wk_sb, bk_sb, xk_ap, "k1")
            k1b = kq_pieces(1, 1, k_sb, wk_sb, bk_sb, xk_ap, "k1")
            k2a = kq_pieces(2, 0, k_sb, wk_sb, bk_sb, xk_ap, "k2")
            k2b = kq_pieces(2, 1, k_sb, wk_sb, bk_sb, xk_ap, "k2")
            k3a = kq_pieces(3, 0, k_sb, wk_sb, bk_sb, xk_ap, "k3")
            k3b = kq_pieces(3, 1, k_sb, wk_sb, bk_sb, xk_ap, "k3")
            q1a = kq_pieces(1, 0, q_sb, wq_sb, bq_sb, xq_ap, "q1")
            q1b = kq_pieces(1, 1, q_sb, wq_sb, bq_sb, xq_ap, "q1")
            q2a = kq_pieces(2, 0, q_sb, wq_sb, bq_sb, xq_ap, "q2")
            q2b = kq_pieces(2, 1, q_sb, wq_sb, bq_sb, xq_ap, "q2")
            q3a = kq_pieces(3, 0, q_sb, wq_sb, bq_sb, xq_ap, "q3")
            q3b = kq_pieces(3, 1, q_sb, wq_sb, bq_sb, xq_ap, "q3")
            v_pc = {}
            for c in range(NCH):
                for th in range(4):
                    pre = None
                    if c == 0 and th == 0:
                        pre = lambda: dma_v(2)
                    if c == 1 and th == 0:
                        pre = lambda: dma_v(3)
                    v_pc[(c, th)] = mk_v(c, th, pre=pre)

            # Interleave so v-block tt lands by slot tt+LAG, k chunk cc by slot
            # 4*cc, q chunk cc by slot 16*cc.
            order = [
                v_pc[(0, 0)], k1a[0], k1a[1],
                v_pc[(0, 1)], k1b[0], k1b[1],
                v_pc[(0, 2)], k2a[0], k2a[1],
                v_pc[(0, 3)], k2b[0], k2b[1],
                v_pc[(1, 0)], k3a[0], k3a[1],
                v_pc[(1, 1)], k3b[0], k3b[1],
                v_pc[(1, 2)], v_pc[(1, 3)],
                q1a[0], q1a[1], q1b[0], q1b[1],
                v_pc[(2, 0)], v_pc[(2, 1)], v_pc[(2, 2)], v_pc[(2, 3)],
                v_pc[(3, 0)], v_pc[(3, 1)], v_pc[(3, 2)], v_pc[(3, 3)],
                q2a[0], q2a[1], q2b[0], q2b[1],
                q3a[0], q3a[1], q3b[0], q3b[1],
            ]
            fillers.extend(order)

            # ---- attention stream ----
            # Global slot stream: chunk c contributes 16 QK/exp slots; PV for
            # (c, tt) is emitted LAG slots later (spilling into the next
            # chunk's slots); normalization at each chunk boundary.
            pts = {}       # (chunk, tt, pair) -> pts tile
            pv_ps = {}     # (chunk, pair) -> PSUM accumulator
            dn_ps = {}     # chunk -> denominator PSUM

            def emit_qk(c, tt):
                csl = ts(c, CH)
                for pm in range(2):
                    st = psum.tile([P, 2, CH], f32, tag=f"qk{pm}", name=f"qk_{c}_{tt}_{pm}")
                    for j in range(2):
                        base = DH * j
                        nc.tensor.matmul(
                            st[:, j, :],
                            k_sb[base : base + DH, pm, ts(tt, P)],
                            q_sb[base : base + DH, pm, csl],
                            start=True, stop=True,
                        )
                    pt = ptpool.tile([P, 2, CH], bf16, tag=f"pt{pm}", name=f"pt_{c}_{tt}_{pm}")
                    nc.scalar.activation(out=pt, in_=st, func=EXP)
                    pts[(c, tt, pm)] = pt

            def emit_pv(c, tt):
                if tt == 0:
                    # open each shared bank with a full-width zeroing matmul
                    # (the only start=True): the per-head chains then ride its
                    # has_written bits with start=False.
                    for pm in range(2):
                        po = psum.tile([P, CH], f32, tag=f"pv{pm}", name=f"pv_{c}_{pm}")
                        pv_ps[(c, pm)] = po
                        nc.tensor.matmul(po, zero_sb[:, 0:P], zero_sb,
                                         start=True, stop=False, skip_group_check=True)
                    dn = psum.tile([P, CH], f32, tag="dn", name=f"dn_{c}")
                    dn_ps[c] = dn
                    nc.tensor.matmul(dn, zero_sb[:, 0:P], zero_sb,
                                     start=True, stop=False, skip_group_check=True)
                for pm in range(2):
                    pt = pts[(c, tt, pm)]
                    po = pv_ps[(c, pm)]
                    for j in range(2):
                        # stop on BOTH chains at the last tile: the group
                        # bookkeeping is per-partition, and each chain's stop
                        # closes only its own partition range.
                        nc.tensor.matmul(
                            po[DH * j : DH * j + DH, :],
                            v_heads[:, tt, 2 * pm + j, :],
                            pt[:, j, :],
                            start=False,
                            stop=(tt == TT - 1),
                            skip_group_check=True,
                        )
                dn = dn_ps[c]
                for pm in range(2):
                    pt = pts[(c, tt, pm)]
                    for j in range(2):
                        h = 2 * pm + j
                        nc.tensor.matmul(
                            dn[32 * h : 32 * h + 1, :],
                            ones_sb,
                            pt[:, j, :],
                            start=False,
                            stop=False,
                            skip_group_check=True,
                            tile_position=(0, 32 * h),
                        )
                if tt == TT - 1:
                    # the denominator chains only touch partitions 0/32/64/96;
                    # a full-width zero-add closes the whole bank's group.
                    nc.tensor.matmul(dn, zero_sb[:, 0:P], zero_sb,
                                     start=False, stop=True, skip_group_check=True)

            def emit_norm(c, tail=False):
                csl = ts(c, CH)
                # fast copies free the PV/denominator banks immediately; the
                # slow reciprocal then runs off the critical path from SBUF.
                tmp = {}
                for pm in range(2):
                    t = spool.tile([P, CH], f32, tag=f"tmp{pm}", name=f"tmp_{c}_{pm}")
                    nc.vector.tensor_copy(out=t, in_=pv_ps[(c, pm)])
                    tmp[pm] = t
                rec = spool.tile([P, CH], f32, tag="rec", name=f"rec_{c}")
                dnc = spool.tile([P, CH], f32, tag="dnc", name=f"dnc_{c}")
                nc.vector.tensor_copy(out=dnc[0:97, :], in_=dn_ps[c][0:97, :])
                nc.vector.reciprocal(out=rec[0:97, :], in_=dnc[0:97, :])
                # move the three off-partition-0 reciprocal rows to partition 0
                # (SBUF->SBUF DMA remaps partitions; gpsimd broadcast reads
                # partition 0 only)
                rrow = [rec[0:1, :]]
                for h in range(1, 4):
                    r = spool.tile([1, CH], f32, tag=f"r{h}", name=f"r_{c}_{h}")
                    nc.sync.dma_start(out=r, in_=rec[32 * h : 32 * h + 1, :])
                    rrow.append(r[0:1, :])
                # HW partition_broadcast only writes from partition base 0:
                # broadcast each head's reciprocal across all 128 partitions
                # and multiply per head with partition-aligned slices.
                for pm in range(2):
                    for j in range(2):
                        b = spool.tile([P, CH], f32, tag=f"bc{pm}{j}", name=f"bc_{c}_{pm}{j}")
                        nc.gpsimd.partition_broadcast(b, rrow[2 * pm + j])
                        sl = slice(DH * j, DH * (j + 1))
                        nc.vector.tensor_mul(out=attn_sb[sl, pm, csl],
                                             in0=tmp[pm][sl, :], in1=b[sl, :])

            slot = [0]

            def run_slot(qk_ct, pv_ct, norm_c=None):
                # chunk 0 drains most of the projection filler (PE-bound);
                # later chunks pace one piece per slot under the ACT pacer.
                # The two slots before each chunk boundary pop nothing so the
                # DVE queue is clear for the bank-freeing copies of emit_norm
                # (a backed-up copy stalls the next chunk's PV clears and, via
                # the in-order PE queue, starves ACT).
                if slot[0] < TT:
                    pop_fillers(2000)
                if qk_ct is not None:
                    emit_qk(*qk_ct)
                if pv_ct is not None:
                    emit_pv(*pv_ct)
                if norm_c is not None:
                    emit_norm(norm_c)
                    for st_ in range(CH // P):
                        for n_ in range(2):
                            fillers.append((1024, (lambda cc, ss, nn: lambda: wo_group(cc, ss, nn))(norm_c, st_, n_)))
                boundary = slot[0] >= TT and slot[0] % TT in (3, 4)
                if not boundary:
                    if slot[0] < TT:
                        pop_fillers(2000 if slot[0] % 2 == 0 else 1)
                    else:
                        pop_fillers(1)
                slot[0] += 1

            stream = []  # (chunk, tt) in QK order
            for c in range(NCH):
                for tt in range(TT):
                    stream.append((c, tt))

            for i, (c, tt) in enumerate(stream):
                pv_i = i - LAG
                pv_ct = stream[pv_i] if pv_i >= 0 else None
                norm_c = None
                if pv_ct is not None and pv_ct[1] == TT - 1 and pv_ct[0] < NCH - 1:
                    norm_c = pv_ct[0]
                run_slot((c, tt), pv_ct, norm_c)

            # tail: trailing PV slots, keep-warm matmuls so the HAM gate stays
            # hot through the final norm chain, then the last chunk's norm and
            # Wo rotated over the now-free banks.
            for i in range(len(stream), len(stream) + LAG):
                pv_ct = stream[i - LAG]
                run_slot(None, pv_ct)
            emit_norm(NCH - 1, tail=True)
            warm2 = psum.tile([P, 2, CH], f32, tag="qk0", name="warm2")
            for w in range(24):
                nc.tensor.matmul(warm2[:, 0, 0:256], zero_sb[:, 0:P], zero_sb[:, 0:256],
                                 start=True, stop=True)
            while fillers:
                pop_fillers(10**9)
            tail_tags = ["mx", "qk1", "pv0", "pv1", "dn"]
            for idx, (st, n) in enumerate((s, n) for s in range(CH // P) for n in range(2)):
                wo_group(NCH - 1, st, n, ptag=tail_tags[idx % len(tail_tags)])

    nc.finalize()
    return nc


def kernel(**inputs):
    global _compiled, last_results
    if _compiled is None:
        _compiled = _build()
    nc = _compiled

    query = np.asarray(inputs["query"], np.float32)
    key = np.asarray(inputs["key"], np.float32)
    value = np.asarray(inputs["value"], np.float32)
    Wq = np.asarray(inputs["Wq"], np.float32)
    Wk = np.asarray(inputs["Wk"], np.float32)
    Wv = np.asarray(inputs["Wv"], np.float32)
    Wo = np.asarray(inputs["Wo"], np.float32)
    bq_f = np.asarray(inputs["bq"], np.float32)
    bk_f = np.asarray(inputs["bk"], np.float32)
    bv_f = np.asarray(inputs["bv"], np.float32)
    bo_f = np.asarray(inputs["bo"], np.float32)

    bf = ml_dtypes.bfloat16
    scale = 1.0 / np.sqrt(np.float32(DH))
    qT = [np.ascontiguousarray(query[b].T).astype(bf) for b in range(B)]
    kT = [np.ascontiguousarray(key[b].T).astype(bf) for b in range(B)]
    vT = [np.ascontiguousarray(value[b].T).astype(bf) for b in range(B)]

    in_maps = []
    for c in range(NCORES):
        b = c // 4
        sh = c % 4
        sl = slice(DHC * sh, DHC * (sh + 1))
        in_maps.append({
            "qT": qT[b], "kT": kT[b], "vT": vT[b],
            "wq": (Wq[:, sl] * scale).astype(bf),
            "wk": np.ascontiguousarray(Wk[:, sl]).astype(bf),
            "wv": np.ascontiguousarray(Wv[:, sl]).astype(bf),
            "wo": np.ascontiguousarray(Wo[sl, :]).astype(bf),
            "bq": np.ascontiguousarray(bq_f[sl]) * scale,
            "bk": np.ascontiguousarray(bk_f[sl]),
            "bv": np.ascontiguousarray(bv_f[sl]),
        })

    res = bass_utils.run_bass_kernel_spmd(nc, in_maps, core_ids=list(range(NCORES)))
    last_results = res

    final = np.empty((B, S, D), np.float32)
    for b in range(B):
        acc = res.results[4 * b]["out"].astype(np.float32)
        for sh in range(1, 4):
            acc = acc + res.results[4 * b + sh]["out"]
        final[b] = acc + bo_f
    return final


# revision 32
# speedup vs baseline: 1.2444x; 1.0402x over previous
"""Multi-head attention (B=2, S=2048, D=1024, H=16, dh=64) on 8 TRN2 NeuronCores.

Sharding: data-parallel over batch (2) x tensor-parallel over heads (4 per core).
Core c handles batch c//4 and heads [4*(c%4), 4*(c%4)+4). Each core computes a
partial output (its heads' contribution through Wo); the host sums the 4 partials
per batch and adds bo.

Schedule: the scalar engine (exp over 16.8M logits at 1 elem/cycle/lane @1.2GHz)
is the pacer, so the kernel is a single fine-grained pipeline that keeps ACT
saturated: per key-tile "slot" the PE issues QK for both head pairs, ACT exps the
previous stage, and the PE fills the rest of the slot with PV (lagging a few
slots), softmax-denominator matmuls, and interleaved k/q/v projection and Wo
output-projection work.

PE-array tiling exploits:
 - QK has contraction K=64 (head dim): the two heads of a pair sit at row tiles
   (0,0) and (64,0) and their matmuls are issued adjacently -> they execute
   concurrently (2x QK).
 - PV has M=64 (value dims): the two heads of a pair write PSUM partitions 0:64
   and 64:128 via col tiles (0,0)/(0,64) -> concurrent (2x PV).
 - Softmax denominators are M=1 matmuls against an all-ones stationary, col-tiled
   at positions (0,0/32/64/96) for the 4 heads -> all four concurrent, one PSUM
   bank, one reciprocal per chunk.

PSUM budget (8 banks): QK stages A/B [P,2,CH] = 4, PV pair accumulators = 2,
denominators = 1, shared projection/Wo bank = 1 (the lead-in alternates over the
not-yet-used attention banks to avoid evacuation stalls).
"""

import sys

if "/opt/trn_rl_repo" not in sys.path:
    sys.path.insert(0, "/opt/trn_rl_repo")

from collections import deque

import ml_dtypes
import numpy as np

import concourse.bass as bass
import concourse.mybir as mybir
import concourse.tile as tile
from concourse import bacc, bass_utils
from concourse.bass import ts

# Problem constants (hardcoded per contract)
B, S, D = 2, 2048, 1024
H, DH = 16, 64            # total heads, head dim
HC = 4                    # heads per core
DHC = HC * DH             # 256 projected dims per core
NCORES = 8
P = 128
CH = 512                  # query-chunk
NCH = S // CH             # 4
TT = S // P               # 16 key tiles
KO = D // P               # 8 contraction tiles for projections
LAG = 6                   # PV trails QK by this many key-tile slots

f32 = mybir.dt.float32
bf16 = mybir.dt.bfloat16
EXP = mybir.ActivationFunctionType.Exp

_compiled = None          # cached nc across calls
last_results = None       # BassKernelResults of the most recent run (for profiling)


def _build():
    nc = bacc.Bacc("TRN2", target_bir_lowering=False, debug=False)

    qT = nc.dram_tensor("qT", [D, S], bf16, kind="ExternalInput")
    kT = nc.dram_tensor("kT", [D, S], bf16, kind="ExternalInput")
    vT = nc.dram_tensor("vT", [D, S], bf16, kind="ExternalInput")
    wq = nc.dram_tensor("wq", [D, DHC], bf16, kind="ExternalInput")
    wk = nc.dram_tensor("wk", [D, DHC], bf16, kind="ExternalInput")
    wv = nc.dram_tensor("wv", [D, DHC], bf16, kind="ExternalInput")
    wo = nc.dram_tensor("wo", [DHC, D], bf16, kind="ExternalInput")
    bq = nc.dram_tensor("bq", [DHC], f32, kind="ExternalInput")
    bk = nc.dram_tensor("bk", [DHC], f32, kind="ExternalInput")
    bv = nc.dram_tensor("bv", [DHC], f32, kind="ExternalInput")
    out = nc.dram_tensor("out", [S, D], bf16, kind="ExternalOutput")

    qT_r = qT.ap().rearrange("(ko p) s -> p ko s", p=P)
    kT_r = kT.ap().rearrange("(ko p) s -> p ko s", p=P)
    vT_r = vT.ap().rearrange("(ko p) s -> p ko s", p=P)

    with tile.TileContext(nc) as tc:
        with (
            tc.tile_pool(name="weights", bufs=1) as wpool,
            tc.tile_pool(name="acts", bufs=1) as apool,
            tc.tile_pool(name="xin", bufs=11) as xpool,
            tc.tile_pool(name="pt", bufs=8) as ptpool,
            tc.tile_pool(name="small", bufs=1) as spool,
            tc.tile_pool(name="outs", bufs=2) as opool,
            tc.tile_pool(name="psum", bufs=1, space="PSUM") as psum,
        ):
            # ---- weights / constants ----
            # DMA emission order follows first-use: k-path, q-path, v-path,
            # biases, then wo (first needed ~20 slots in).
            wq_sb = wpool.tile([P, KO, DHC], bf16, tag="wq")
            wk_sb = wpool.tile([P, KO, DHC], bf16, tag="wk")
            wv_sb = wpool.tile([P, KO, DHC], bf16, tag="wv")
            wo_sb = wpool.tile([P, DHC // P, D], bf16, tag="wo")
            nc.sync.dma_start(out=wk_sb, in_=wk.ap().rearrange("(ko p) m -> p ko m", p=P))
            nc.scalar.dma_start(out=wq_sb, in_=wq.ap().rearrange("(ko p) m -> p ko m", p=P))
            nc.gpsimd.dma_start(out=wv_sb, in_=wv.ap().rearrange("(ko p) m -> p ko m", p=P))
            bq_sb = wpool.tile([P, 2], f32, tag="bq")
            bk_sb = wpool.tile([P, 2], f32, tag="bk")
            nc.gpsimd.dma_start(out=bq_sb, in_=bq.ap().rearrange("(mo p) -> p mo", p=P))
            nc.gpsimd.dma_start(out=bk_sb, in_=bk.ap().rearrange("(mo p) -> p mo", p=P))
            bv_row = wpool.tile([P, DHC], f32, tag="bv_row")
            nc.gpsimd.dma_start(out=bv_row[0:1, :], in_=bv.ap().rearrange("(a d) -> a d", a=1))
            bv_bc = wpool.tile([P, DHC], f32, tag="bv_bc")
            nc.gpsimd.partition_broadcast(bv_bc, bv_row[0:1, :])
            ones_sb = wpool.tile([P, 1], bf16, tag="ones")
            nc.vector.memset(ones_sb, 1.0)
            # zero operands for the PSUM-clearing matmul that opens each shared
            # accumulation bank (PV pairs / denominators share banks between
            # chains, so only the clear carries start=True).
            zero_sb = wpool.tile([P, CH], bf16, tag="zero")
            nc.vector.memset(zero_sb, 0.0)

            # ---- activations ----
            # q^T/k^T: [P, pair, S]; projected dim r of pair m sits at
            # (r % 128, m): head 2m at partitions 0:64, head 2m+1 at 64:128.
            q_sb = apool.tile([P, 2, S], bf16, tag="q")
            k_sb = apool.tile([P, 2, S], bf16, tag="k")
            # v natural: [P(key in tile), tt, h*dh]
            v_sb = apool.tile([P, TT, DHC], bf16, tag="v")
            v_heads = v_sb[:, :, :].rearrange("p tt (h c) -> p tt h c", c=DH)
            # attn^T accumulator, same layout as q_sb
            attn_sb = apool.tile([P, 2, S], bf16, tag="attn")

            # ---- DMA + projection-group emitters ----
            xk = [None] * NCH
            xq = [None] * NCH
            xv = [None] * NCH

            def dma_k(c, eng=None):
                xk[c] = xpool.tile([P, KO, CH], bf16, tag="x", name=f"xk{c}")
                (eng or nc.sync).dma_start(out=xk[c], in_=kT_r[:, :, ts(c, CH)])

            def dma_q(c, eng=None):
                xq[c] = xpool.tile([P, KO, CH], bf16, tag="x", name=f"xq{c}")
                (eng or nc.sync).dma_start(out=xq[c], in_=qT_r[:, :, ts(c, CH)])

            def dma_v(c, eng=None):
                xv[c] = xpool.tile([P, KO, CH], bf16, tag="x", name=f"xv{c}")
                (eng or nc.sync).dma_start(out=xv[c], in_=vT_r[:, :, ts(c, CH)])

            def xk_ap(c):
                return xk[c]

            def xq_ap(c):
                return xq[c]

            def kq_group(c, m, dst, w_sb, b_sb, x, ptag, uid):
                # one projection group: [P, CH] of pair m for chunk c
                ps = psum.tile([P, CH], f32, tag=ptag, name=f"pj_{uid}_{m}")
                for ko in range(KO):
                    nc.tensor.matmul(ps, w_sb[:, ko, ts(m, P)], x[:, ko, :],
                                     start=(ko == 0), stop=(ko == KO - 1))
                nc.vector.tensor_scalar_add(out=dst[:, m, ts(c, CH)], in0=ps,
                                            scalar1=b_sb[:, m : m + 1])

            def v_group(c, th):
                tt = c * (CH // P) + th
                ps = psum.tile([P, DHC], f32, tag="mx", name=f"pv_proj_{tt}")
                for ko in range(KO):
                    nc.tensor.matmul(ps, xv[c][:, ko, ts(th, P)], wv_sb[:, ko, :],
                                     start=(ko == 0), stop=(ko == KO - 1))
                nc.vector.tensor_add(out=v_sb[:, tt, :], in0=ps, in1=bv_bc)

            def wo_group(c, st, n, ptag="mx", evac=None):
                stg = c * (CH // P) + st
                pw = psum.tile([P, 512], f32, tag=ptag, name=f"wo_{stg}_{n}")
                for ko in range(2):
                    nc.tensor.matmul(pw, attn_sb[:, ko, ts(stg, P)],
                                     wo_sb[:, ko, ts(n, 512)],
                                     start=(ko == 0), stop=(ko == 1))
                ot = opool.tile([P, 512], bf16, tag="ot", name=f"ot_{stg}_{n}")
                if evac is None:
                    nc.vector.tensor_copy(out=ot, in_=pw)
                else:
                    evac(ot, pw)
                nc.sync.dma_start(out=out.ap()[ts(stg, P), ts(n, 512)], in_=ot)

            # ---- filler queue: (cost_cycles, emit_fn) ----
            fillers = deque()

            def pop_fillers(budget):
                spent = 0
                while fillers and spent < budget:
                    cost, fn = fillers.popleft()
                    fn()
                    spent += cost

            # ---- lead-in ----
            # All x DMAs kick up-front in deadline order on the two HWDGE
            # queues (~190GB/s each measured): sync carries wk + k + v, the
            # scalar queue (idle pre-stream) carries wq + q. SWDGE (gpsimd)
            # only moves the small weights so it never contends for HBM in
            # the critical window.
            dma_k(0)
            dma_q(0, eng=nc.scalar)
            dma_k(1)
            dma_q(1, eng=nc.scalar)
            dma_v(0)
            dma_k(2)
            dma_k(3)
            dma_v(1)
            dma_q(2, eng=nc.gpsimd)
            dma_q(3, eng=nc.gpsimd)
            # wo is first needed ~20 slots in; keep it behind the x DMAs.
            nc.gpsimd.dma_start(out=wo_sb, in_=wo.ap().rearrange("(ko p) n -> p ko n", p=P))
            # Warm the PE HAM clock gate during the initial DMA wait: ~4.5us of
            # dependency-free matmuls (zero x zero) so the real lead-in runs at
            # 2.4GHz instead of 1.2.
            warm_ps = psum.tile([P, 2, CH], f32, tag="qk0", name="warm")
            for w in range(10):
                nc.tensor.matmul(warm_ps[:, 0, :], zero_sb[:, 0:P], zero_sb,
                                 start=True, stop=True)
            # k/q for chunk 0 on the four attention banks (not yet used) so the
            # four groups don't serialize on one bank's evacuation.
            kq_group(0, 0, k_sb, wk_sb, bk_sb, xk[0], "mx", "k0")
            kq_group(0, 1, k_sb, wk_sb, bk_sb, xk[0], "dn", "k0")
            kq_group(0, 0, q_sb, wq_sb, bq_sb, xq[0], "pv0", "q0")
            kq_group(0, 1, q_sb, wq_sb, bq_sb, xq[0], "pv1", "q0")

            # Remaining projections become filler pieces (~2048 PE cycles each):
            # k/q groups split into two half-groups sharing one PSUM tile.
            def mk_kq_halves(c, m, dst, w_sb, b_sb, xap, uid, pre=None):
                state = {}

                def fn1():
                    if pre is not None:
                        pre()
                    ps = psum.tile([P, CH], f32, tag="mx", name=f"pj_{uid}_{m}")
                    state["ps"] = ps
                    x = xap(c)
                    for ko in range(KO // 2):
                        nc.tensor.matmul(ps, w_sb[:, ko, ts(m, P)], x[:, ko, :],
                                         start=(ko == 0), stop=False)

                def fn2():
                    ps = state["ps"]
                    x = xap(c)
                    for ko in range(KO // 2, KO):
                        nc.tensor.matmul(ps, w_sb[:, ko, ts(m, P)], x[:, ko, :],
                                         start=False, stop=(ko == KO - 1))
                    nc.vector.tensor_scalar_add(out=dst[:, m, ts(c, CH)], in0=ps,
                                                scalar1=b_sb[:, m : m + 1])

                return (2048, fn1), (2048, fn2)

            def mk_v(c, th, pre=None):
                def fn():
                    if pre is not None:
                        pre()
                    v_group(c, th)
                return (2048, fn)

            def kq_pieces(c, m, dst, w_sb, b_sb, xap, uid, pre=None):
                return mk_kq_halves(c, m, dst, w_sb, b_sb, xap, uid, pre=pre)

            k1a = kq_pieces(1, 0, k_sb, wk_sb, bk_sb, xk_ap, "k1")
            k1b = kq_pieces(1, 1, k_sb, wk_sb, bk_sb, xk_ap, "k1")
            k2a = kq_pieces(2, 0, k_sb, wk_sb, bk_sb, xk_ap, "k2")
            k2b = kq_pieces(2, 1, k_sb, wk_sb, bk_sb, xk_ap, "k2")
            k3a = kq_pieces(3, 0, k_sb, wk_sb, bk_sb, xk_ap, "k3")
            k3b = kq_pieces(3, 1, k_sb, wk_sb, bk_sb, xk_ap, "k3")
            q1a = kq_pieces(1, 0, q_sb, wq_sb, bq_sb, xq_ap, "q1")
            q1b = kq_pieces(1, 1, q_sb, wq_sb, bq_sb, xq_ap, "q1")
            q2a = kq_pieces(2, 0, q_sb, wq_sb, bq_sb, xq_ap, "q2")
            q2b = kq_pieces(2, 1, q_sb, wq_sb, bq_sb, xq_ap, "q2")
            q3a = kq_pieces(3, 0, q_sb, wq_sb, bq_sb, xq_ap, "q3")
            q3b = kq_pieces(3, 1, q_sb, wq_sb, bq_sb, xq_ap, "q3")
            v_pc = {}
            for c in range(NCH):
                for th in range(4):
                    pre = None
                    if c == 0 and th == 0:
                        pre = lambda: dma_v(2)
                    if c == 1 and th == 0:
                        pre = lambda: dma_v(3)
                    v_pc[(c, th)] = mk_v(c, th, pre=pre)

            # Interleave so v-block tt lands by slot tt+LAG, k chunk cc by slot
            # 4*cc, q chunk cc by slot 16*cc.
            order = [
                k1a[0], k1a[1], k1b[0], k1b[1],
                v_pc[(0, 0)], v_pc[(0, 1)],
                k2a[0], k2a[1],
                v_pc[(0, 2)], v_pc[(0, 3)],
                k2b[0], k2b[1],
                k3a[0], k3a[1],
                v_pc[(1, 0)], v_pc[(1, 1)],
                k3b[0], k3b[1],
                v_pc[(1, 2)], v_pc[(1, 3)],
                q1a[0], q1a[1], q1b[0], q1b[1],
                v_pc[(2, 0)], v_pc[(2, 1)], v_pc[(2, 2)], v_pc[(2, 3)],
                v_pc[(3, 0)], v_pc[(3, 1)], v_pc[(3, 2)], v_pc[(3, 3)],
                q2a[0], q2a[1], q2b[0], q2b[1],
                q3a[0], q3a[1], q3b[0], q3b[1],
            ]
            fillers.extend(order)

            # ---- attention stream ----
            # Global slot stream: chunk c contributes 16 QK/exp slots; PV for
            # (c, tt) is emitted LAG slots later (spilling into the next
            # chunk's slots); normalization at each chunk boundary.
            pts = {}       # (chunk, tt, pair) -> pts tile
            pv_ps = {}     # (chunk, pair) -> PSUM accumulator
            dn_ps = {}     # chunk -> denominator PSUM

            def emit_qk(c, tt):
                csl = ts(c, CH)
                for pm in range(2):
                    st = psum.tile([P, 2, CH], f32, tag=f"qk{pm}", name=f"qk_{c}_{tt}_{pm}")
                    for j in range(2):
                        base = DH * j
                        nc.tensor.matmul(
                            st[:, j, :],
                            k_sb[base : base + DH, pm, ts(tt, P)],
                            q_sb[base : base + DH, pm, csl],
                            start=True, stop=True,
                        )
                    pt = ptpool.tile([P, 2, CH], bf16, tag=f"pt{pm}", name=f"pt_{c}_{tt}_{pm}")
                    nc.scalar.activation(out=pt, in_=st, func=EXP)
                    pts[(c, tt, pm)] = pt

            def emit_pv(c, tt):
                if tt == 0:
                    # open each shared bank with a full-width zeroing matmul
                    # (the only start=True): the per-head chains then ride its
                    # has_written bits with start=False.
                    for pm in range(2):
                        po = psum.tile([P, CH], f32, tag=f"pv{pm}", name=f"pv_{c}_{pm}")
                        pv_ps[(c, pm)] = po
                        nc.tensor.matmul(po, zero_sb[:, 0:P], zero_sb,
                                         start=True, stop=False, skip_group_check=True)
                    dn = psum.tile([P, CH], f32, tag="dn", name=f"dn_{c}")
                    dn_ps[c] = dn
                    nc.tensor.matmul(dn, zero_sb[:, 0:P], zero_sb,
                                     start=True, stop=False, skip_group_check=True)
                for pm in range(2):
                    pt = pts[(c, tt, pm)]
                    po = pv_ps[(c, pm)]
                    for j in range(2):
                        # stop on BOTH chains at the last tile: the group
                        # bookkeeping is per-partition, and each chain's stop
                        # closes only its own partition range.
                        nc.tensor.matmul(
                            po[DH * j : DH * j + DH, :],
                            v_heads[:, tt, 2 * pm + j, :],
                            pt[:, j, :],
                            start=False,
                            stop=(tt == TT - 1),
                            skip_group_check=True,
                        )
                dn = dn_ps[c]
                for pm in range(2):
                    pt = pts[(c, tt, pm)]
                    for j in range(2):
                        h = 2 * pm + j
                        nc.tensor.matmul(
                            dn[32 * h : 32 * h + 1, :],
                            ones_sb,
                            pt[:, j, :],
                            start=False,
                            stop=False,
                            skip_group_check=True,
                            tile_position=(0, 32 * h),
                        )
                if tt == TT - 1:
                    # the denominator chains only touch partitions 0/32/64/96;
                    # a full-width zero-add closes the whole bank's group.
                    nc.tensor.matmul(dn, zero_sb[:, 0:P], zero_sb,
                                     start=False, stop=True, skip_group_check=True)

            def emit_norm(c, tail=False):
                csl = ts(c, CH)
                # fast copies free the PV/denominator banks immediately; the
                # slow reciprocal then runs off the critical path from SBUF.
                tmp = {}
                for pm in range(2):
                    t = spool.tile([P, CH], f32, tag=f"tmp{pm}", name=f"tmp_{c}_{pm}")
                    nc.vector.tensor_copy(out=t, in_=pv_ps[(c, pm)])
                    tmp[pm] = t
                rec = spool.tile([P, CH], f32, tag="rec", name=f"rec_{c}")
                dnc = spool.tile([P, CH], f32, tag="dnc", name=f"dnc_{c}")
                nc.vector.tensor_copy(out=dnc[0:97, :], in_=dn_ps[c][0:97, :])
                nc.vector.reciprocal(out=rec[0:97, :], in_=dnc[0:97, :])
                # move the three off-partition-0 reciprocal rows to partition 0
                # (SBUF->SBUF DMA remaps partitions; gpsimd broadcast reads
                # partition 0 only)
                rrow = [rec[0:1, :]]
                for h in range(1, 4):
                    r = spool.tile([1, CH], f32, tag=f"r{h}", name=f"r_{c}_{h}")
                    nc.sync.dma_start(out=r, in_=rec[32 * h : 32 * h + 1, :])
                    rrow.append(r[0:1, :])
                # HW partition_broadcast only writes from partition base 0:
                # broadcast each head's reciprocal across all 128 partitions
                # and multiply per head with partition-aligned slices.
                for pm in range(2):
                    for j in range(2):
                        b = spool.tile([P, CH], f32, tag=f"bc{pm}{j}", name=f"bc_{c}_{pm}{j}")
                        nc.gpsimd.partition_broadcast(b, rrow[2 * pm + j])
                        sl = slice(DH * j, DH * (j + 1))
                        nc.vector.tensor_mul(out=attn_sb[sl, pm, csl],
                                             in0=tmp[pm][sl, :], in1=b[sl, :])

            slot = [0]

            def run_slot(qk_ct, pv_ct, norm_c=None):
                # chunk 0 drains most of the projection filler (PE-bound);
                # later chunks pace one piece per slot under the ACT pacer.
                # The two slots before each chunk boundary pop nothing so the
                # DVE queue is clear for the bank-freeing copies of emit_norm
                # (a backed-up copy stalls the next chunk's PV clears and, via
                # the in-order PE queue, starves ACT).
                if slot[0] < TT:
                    pop_fillers(2000)
                if qk_ct is not None:
                    emit_qk(*qk_ct)
                if pv_ct is not None:
                    emit_pv(*pv_ct)
                if norm_c is not None:
                    emit_norm(norm_c)
                    for st_ in range(CH // P):
                        for n_ in range(2):
                            fillers.append((1024, (lambda cc, ss, nn: lambda: wo_group(cc, ss, nn))(norm_c, st_, n_)))
                boundary = slot[0] >= TT and slot[0] % TT in (3, 4)
                if not boundary:
                    if slot[0] < TT:
                        pop_fillers(2000 if slot[0] % 2 == 0 else 1)
                    else:
                        pop_fillers(1)
                slot[0] += 1

            stream = []  # (chunk, tt) in QK order
            for c in range(NCH):
                for tt in range(TT):
                    stream.append((c, tt))

            for i, (c, tt) in enumerate(stream):
                pv_i = i - LAG
                pv_ct = stream[pv_i] if pv_i >= 0 else None
                norm_c = None
                if pv_ct is not None and pv_ct[1] == TT - 1 and pv_ct[0] < NCH - 1:
                    norm_c = pv_ct[0]
                run_slot((c, tt), pv_ct, norm_c)

            # tail: trailing PV slots, keep-warm matmuls so the HAM gate stays
            # hot through the final norm chain, then the last chunk's norm and
            # Wo rotated over the now-free banks.
            for i in range(len(stream), len(stream) + LAG):
                pv_ct = stream[i - LAG]
                run_slot(None, pv_ct)
            emit_norm(NCH - 1, tail=True)
            warm2 = psum.tile([P, 2, CH], f32, tag="qk0", name="warm2")
            for w in range(48):
                nc.tensor.matmul(warm2[:, 0, 0:256], zero_sb[:, 0:P], zero_sb[:, 0:256],
                                 start=True, stop=True)
            while fillers:
                pop_fillers(10**9)
            tail_tags = ["mx", "qk1", "pv0", "pv1", "dn"]

            def act_evac(ot, pw):
                nc.scalar.activation(out=ot, in_=pw,
                                     func=mybir.ActivationFunctionType.Copy)

            for idx, (st, n) in enumerate((s, n) for s in range(CH // P) for n in range(2)):
                wo_group(NCH - 1, st, n, ptag=tail_tags[idx % len(tail_tags)],
                         evac=act_evac if idx % 2 else None)

    nc.finalize()
    return nc


def kernel(**inputs):
    global _compiled, last_results
    if _compiled is None:
        _compiled = _build()
    nc = _compiled

    query = np.asarray(inputs["query"], np.float32)
    key = np.asarray(inputs["key"], np.float32)
    value = np.asarray(inputs["value"], np.float32)
    Wq = np.asarray(inputs["Wq"], np.float32)
    Wk = np.asarray(inputs["Wk"], np.float32)
    Wv = np.asarray(inputs["Wv"], np.float32)
    Wo = np.asarray(inputs["Wo"], np.float32)
    bq_f = np.asarray(inputs["bq"], np.float32)
    bk_f = np.asarray(inputs["bk"], np.float32)
    bv_f = np.asarray(inputs["bv"], np.float32)
    bo_f = np.asarray(inputs["bo"], np.float32)

    bf = ml_dtypes.bfloat16
    scale = 1.0 / np.sqrt(np.float32(DH))
    qT = [np.ascontiguousarray(query[b].T).astype(bf) for b in range(B)]
    kT = [np.ascontiguousarray(key[b].T).astype(bf) for b in range(B)]
    vT = [np.ascontiguousarray(value[b].T).astype(bf) for b in range(B)]

    in_maps = []
    for c in range(NCORES):
        b = c // 4
        sh = c % 4
        sl = slice(DHC * sh, DHC * (sh + 1))
        in_maps.append({
            "qT": qT[b], "kT": kT[b], "vT": vT[b],
            "wq": (Wq[:, sl] * scale).astype(bf),
            "wk": np.ascontiguousarray(Wk[:, sl]).astype(bf),
            "wv": np.ascontiguousarray(Wv[:, sl]).astype(bf),
            "wo": np.ascontiguousarray(Wo[sl, :]).astype(bf),
            "bq": np.ascontiguousarray(bq_f[sl]) * scale,
            "bk": np.ascontiguousarray(bk_f[sl]),
            "bv": np.ascontiguousarray(bv_f[sl]),
        })

    res = bass_utils.run_bass_kernel_spmd(nc, in_maps, core_ids=list(range(NCORES)))
    last_results = res

    final = np.empty((B, S, D), np.float32)
    for b in range(B):
        acc = res.results[4 * b]["out"].astype(np.float32)
        for sh in range(1, 4):
            acc = acc + res.results[4 * b + sh]["out"]
        final[b] = acc + bo_f
    return final
